# revision 15
# baseline (speedup 1.0000x reference)
"""Block-sparse multi-head attention on 8 Trainium2 NeuronCores.

Problem: y = proj(softmax(mask(q @ k^T / sqrt(hd))) @ v) for
B=2, S=2048, D=1024, H=16 heads, block size 128, with a [16,16] boolean
block mask (True = masked) applied to strictly-upper (k-block > q-block)
blocks.

Sharding: batch x head-group. Core c handles batch c//4 and heads
[4*(c%4), 4*(c%4)+4). No collectives: the host pre-slices inputs
(including pre-transposing x to x^T so the device never transposes) and
sums the 4 per-batch partial projection outputs on the way out.

Device dataflow (per core, matmuls in float32r = full PE rate, ~1e-4 rel):
  qk^T [512,2048]  = w_qk^T @ x^T        (q tiles [q0;q1],[q2;q3]; k tiles
                                          zero-padded per head to K=128 --
                                          K=64 matmuls measure ~2x slower)
  v    [2048,260]  = x^T.T @ w_v_ext     (65 cols/head: 64 v cols + a ones col)
  per head, per k-block ik (mask-specialized at trace time, with the PV
  matmuls software-pipelined one ik behind the QK/exp stage so the PE never
  stalls on ScalarE's exp):
    S^T  [128,q]   = kpad_ik @ q^T       (only visible q runs)
    P~^T [128,q]   = exp(S^T / 8)        (ScalarE; no max subtraction needed:
                                          scores ~ N(0,1), exp is safe in fp32)
    acc  [65,q]   += v_ik_ext^T @ P~^T   (PSUM; row 64 = softmax denominator
                                          via the ones column)
  normalize: acc[0:64] * (1/denom). The denom row is reshaped to [128,16] by
  an SBUF->SBUF DMA (a [1,2048] single-lane reciprocal costs ~13us; the
  reshaped one ~0.2us), reciprocal'd, reshaped back, broadcast across
  partitions on the otherwise-idle GpSimd engine, and multiplied in.
  out  [2048,1024] partial = attn^T.T @ w_proj_slice  (K=128 head pairs,
  odd heads repacked to partitions 64..127 by an SBUF->SBUF DMA)
"""

import numpy as np

import concourse.mybir as mybir
import concourse.tile as tile
from concourse import bacc
from concourse.bass_utils import run_bass_kernel_spmd

B, S, D, H = 2, 2048, 1024, 16
HD = 64          # head dim
BS = 128         # mask block size
NB = S // BS     # 16 blocks per axis
HPC = 4          # heads per core
N_CORES = 8
SCALE = HD ** -0.5

F32 = mybir.dt.float32
F32R = mybir.dt.float32r
EXP = mybir.ActivationFunctionType.Exp

_program_cache: dict[bytes, object] = {}


def _plan_runs(vis, last_vis, ik):
    """Contiguous visible q-block runs for k-block ik, each within one
    512-col PSUM bank window. The PSUM stop flag is sim-only bookkeeping
    (nothing on HW); runs are merged across stop boundaries and the PV
    matmuls pass skip_group_check."""
    runs = []
    jq = 0
    while jq < NB:
        if not vis[jq][ik]:
            jq += 1
            continue
        start = jq
        while jq + 1 < NB and vis[jq + 1][ik] and (jq + 1) % 4 != 0:
            jq += 1
        stopf = any(last_vis[b] == ik for b in range(start, jq + 1))
        runs.append((start, jq - start + 1, stopf))
        jq += 1
    return runs


def _build_program(mask: np.ndarray):
    """Build + compile the (SPMD, mask-specialized) Bass program."""
    # vis[jq][ik]: may q-block jq attend to k-block ik?
    vis = [
        [ik <= jq or not bool(mask[jq, ik]) for ik in range(NB)] for jq in range(NB)
    ]
    last_vis = [max(ik for ik in range(NB) if vis[jq][ik]) for jq in range(NB)]

    nc = bacc.Bacc("TRN2", target_bir_lowering=False, debug=False,
                   num_devices=N_CORES)
    xT_d = nc.dram_tensor("xT", [D, S], F32R, kind="ExternalInput")
    wqk_d = nc.dram_tensor("wqk", [D, HPC * 2 * HD], F32R, kind="ExternalInput")
    wv_d = nc.dram_tensor("wv", [D, HPC * (HD + 1)], F32R, kind="ExternalInput")
    wpr_d = nc.dram_tensor("wpr", [HPC * HD, D], F32R, kind="ExternalInput")
    out_d = nc.dram_tensor("out", [S, D], F32, kind="ExternalOutput")

    KT = D // 128    # 8 k-tiles over the embedding dim
    MT = S // 128    # 16 seq tiles
    VW = HPC * (HD + 1)  # 260

    with tile.TileContext(nc) as tc:
        with tc.tile_pool(name="persist", bufs=1) as pp:
            wpr_t = [pp.tile([128, D], F32R, tag=f"wpr{k}", name=f"wpr{k}")
                     for k in range(2)]
            # q_t[0]=[qT_h0;qT_h1], q_t[1]=[qT_h2;qT_h3]
            q_t = [pp.tile([128, S], F32R, tag=f"q{j}", name=f"q{j}")
                   for j in range(2)]
            # kpad_t[h]: head h's kT in its own 64 partitions, 0 elsewhere,
            # so QK can contract over K=128 (the zero rows contribute 0
            # against the other head's q rows in q_t[h//2])
            kpad_t = [pp.tile([128, S], F32R, tag=f"kp{h}", name=f"kp{h}")
                      for h in range(HPC)]
            v_t = [pp.tile([128, VW], F32R, tag=f"v{m}", name=f"v{m}")
                   for m in range(MT)]
            # head-pair attn tiles for the K=128 projection
            attn_t = [pp.tile([128, S], F32R, tag=f"attn{i}", name=f"attn{i}")
                      for i in range(2)]
            onec_t = pp.tile([128, 1], F32, tag="onec", name="onec")

            nc.vector.memset(onec_t[:], 1.0)
            zsrc_t = pp.tile([64, S], F32, tag="zsrc", name="zsrc")
            nc.vector.memset(zsrc_t[:], 0.0)
            for h in range(HPC):
                z0, z1 = (64, 128) if h % 2 == 0 else (0, 64)
                # f32 -> f32r copy is a rounding producer (plain memset on an
                # f32r tile fails both the ISA check and the f32r verifier)
                nc.vector.tensor_copy(kpad_t[h][z0:z1, :], zsrc_t[:])

            # ---- load x^T and weight slices; project to qk^T and v ----
            with tc.tile_pool(name="inpool", bufs=1) as ip, \
                 tc.tile_pool(name="psB", bufs=3, space="PSUM") as pbp, \
                 tc.tile_pool(name="psC", bufs=2, space="PSUM") as pcp:
                xT_t = [ip.tile([128, S], F32R, tag=f"xT{k}", name=f"xT{k}")
                        for k in range(KT)]
                wqk_t = [ip.tile([128, HPC * 2 * HD], F32R, tag=f"wqk{k}",
                                 name=f"wqk{k}") for k in range(KT)]
                wv_t = [ip.tile([128, VW], F32R, tag=f"wv{k}", name=f"wv{k}")
                        for k in range(KT)]
                # x^T and w_qk pace stage B's first accumulation chain; wv and
                # w_proj are needed later (C / proj), so they queue behind
                for k in range(KT):
                    nc.sync.dma_start(out=wqk_t[k][:], in_=wqk_d[k * 128:(k + 1) * 128, :])
                    nc.sync.dma_start(out=xT_t[k][:], in_=xT_d[k * 128:(k + 1) * 128, :])
                for k in range(KT):
                    nc.sync.dma_start(out=wv_t[k][:], in_=wv_d[k * 128:(k + 1) * 128, :])
                for k in range(2):
                    nc.sync.dma_start(out=wpr_t[k][:], in_=wpr_d[k * 128:(k + 1) * 128, :])

                # qk^T: per column tile j, two half-tiles of [128, 1024].
                # j=0,1: q head pairs; j=2,3: k head pairs (split to kpad).
                # Order q01, k01, q23, k23 so head 0/1 attention unblocks first.
                for j in (0, 2, 1, 3):
                    for half in range(2):
                        pb = pbp.tile([128, 1024], F32, tag="pb",
                                      name=f"pb{j}{half}")
                        for k in range(KT):
                            lhsT = wqk_t[k][:, j * 128:(j + 1) * 128]
                            for c in range(2):
                                cs = half * 1024 + c * 512
                                nc.tensor.matmul(
                                    pb[:, c * 512:(c + 1) * 512], lhsT,
                                    xT_t[k][:, cs:cs + 512],
                                    start=(k == 0), stop=(k == KT - 1))
                        hs = half * 1024
                        if j < 2:
                            dst = q_t[j][:, hs:hs + 1024]
                            if j % 2 == 0:
                                nc.vector.tensor_copy(dst, pb[:])
                            else:
                                nc.scalar.copy(dst, pb[:])
                        else:
                            heads = (0, 1) if j == 2 else (2, 3)
                            nc.vector.tensor_copy(
                                kpad_t[heads[0]][0:64, hs:hs + 1024], pb[0:64, :])
                            nc.scalar.copy(
                                kpad_t[heads[1]][64:128, hs:hs + 1024], pb[64:128, :])

                # v_ext: natural layout [seq, 260]
                for m in range(MT):
                    pc = pcp.tile([128, VW], F32, tag="pc", name=f"pc{m}")
                    for k in range(KT):
                        nc.tensor.matmul(
                            pc[:], xT_t[k][:, m * 128:(m + 1) * 128], wv_t[k][:],
                            start=(k == 0), stop=(k == KT - 1))
                    nc.vector.tensor_copy(v_t[m][:], pc[:])
                    for j in range(HPC):
                        oc = j * (HD + 1) + HD
                        nc.vector.tensor_copy(v_t[m][:, oc:oc + 1], onec_t[:])

            # ---- attention (mask-specialized) + normalize ----
            with tc.tile_pool(name="atpool", bufs=1) as ap, \
                 tc.tile_pool(name="psA", bufs=1, space="PSUM") as pap, \
                 tc.tile_pool(name="psS", bufs=2, space="PSUM") as psp:
                for j in range(HPC):
                    qtile = q_t[j // 2]
                    pa = pap.tile([65, S], F32, tag="pa", name=f"pa{j}")
                    # software pipeline: PV trails QK/exp by one k-block so
                    # the PE never waits on ScalarE's exp latency. Runs are
                    # grouped into 1024-col window pairs sharing one exp op
                    # (the ~350-cycle ACT per-op overhead dominates at run
                    # granularity; exp over unwritten gap columns is safe --
                    # stale PSUM holds bounded pre-softmax scores).
                    pending = []
                    for ik in range(NB):
                        lhsT_k = kpad_t[j][:, ik * 128:(ik + 1) * 128]
                        lhsT_v = v_t[ik][:, j * (HD + 1):(j + 1) * (HD + 1)]
                        new_pending = []
                        runs = _plan_runs(vis, last_vis, ik)
                        for g in range(2):
                            gb = g * 1024
                            gruns = [r for r in runs if gb <= r[0] * 128 < gb + 1024]
                            if not gruns:
                                continue
                            lo = min(r[0] * 128 for r in gruns) - gb
                            hi = max((r[0] + r[1]) * 128 for r in gruns) - gb
                            stg = psp.tile([128, 1024], F32, tag="st",
                                           name=f"st{j}_{ik}_{g}")
                            for (qb0, nbk, stopf) in gruns:
                                qs, qlen = qb0 * 128, nbk * 128
                                nc.tensor.matmul(
                                    stg[:, qs - gb:qs - gb + qlen], lhsT_k,
                                    qtile[:, qs:qs + qlen],
                                    start=True, stop=True)
                            ptg = ap.tile([128, 1024], F32R, tag="pt", bufs=4,
                                          name=f"pt{j}_{ik}_{g}")
                            nc.scalar.activation(ptg[:, lo:hi], stg[:, lo:hi],
                                                 EXP, scale=SCALE)
                            for (qb0, nbk, stopf) in gruns:
                                qs, qlen = qb0 * 128, nbk * 128
                                new_pending.append(
                                    (lhsT_v, ptg, gb, qs, qlen, ik == 0, stopf))
                        for (lv, ptg, gb, qs, qlen, startf, stopf) in pending:
                            nc.tensor.matmul(pa[0:65, qs:qs + qlen], lv,
                                             ptg[:, qs - gb:qs - gb + qlen],
                                             start=startf, stop=stopf,
                                             skip_group_check=True)
                        pending = new_pending
                    for (lv, ptg, gb, qs, qlen, startf, stopf) in pending:
                        nc.tensor.matmul(pa[0:65, qs:qs + qlen], lv,
                                         ptg[:, qs - gb:qs - gb + qlen],
                                         start=startf, stop=stopf,
                                         skip_group_check=True)
                    # Drain pa fast (so the next head's PV can reuse its
                    # banks), then normalize off the PE critical path.
                    if j % 2 == 0:
                        dst_tile = attn_t[j // 2]
                    else:
                        dst_tile = ap.tile([64, S], F32R, tag="oddh", bufs=1,
                                           name=f"oddh{j}")
                    dnr = ap.tile([65, S], F32, tag="dnr", bufs=2, name=f"dnr{j}")
                    nc.vector.tensor_copy(dnr[64:65, :], pa[64:65, :])
                    nc.vector.tensor_copy(dst_tile[0:64, :], pa[0:64, :])
                    d16 = ap.tile([128, NB], F32, tag="d16", bufs=2,
                                  name=f"d16_{j}")
                    nc.sync.dma_start(out=d16[:], in_=dnr[64:65, :])
                    nc.vector.reciprocal(d16[:], d16[:])
                    r0 = ap.tile([1, S], F32, tag="r0", bufs=2, name=f"r0_{j}")
                    nc.sync.dma_start(out=r0[:], in_=d16[:])
                    dbc = ap.tile([64, S], F32, tag="dbc", bufs=2,
                                  name=f"dbc{j}")
                    nc.gpsimd.partition_broadcast(dbc[:], r0[:])
                    nc.vector.tensor_mul(dst_tile[0:64, :], dst_tile[0:64, :],
                                         dbc[:])
                    if j % 2 == 1:
                        # engines can't shift partitions; DMA packs the odd
                        # head into rows 64..127 of the pair tile (on the
                        # GpSimd SWDGE queue to keep Sync free for output)
                        nc.gpsimd.dma_start(out=attn_t[j // 2][64:128, :],
                                            in_=dst_tile[:])

            # ---- output projection (partial; host sums across head groups) ----
            with tc.tile_pool(name="opool", bufs=2) as op, \
                 tc.tile_pool(name="psO", bufs=2, space="PSUM") as pop:
                for m in range(MT):
                    po = pop.tile([128, D], F32, tag="po", name=f"po{m}")
                    for kt in range(2):
                        lhsT = attn_t[kt][:, m * 128:(m + 1) * 128]
                        for c in range(2):
                            nc.tensor.matmul(
                                po[:, c * 512:(c + 1) * 512], lhsT,
                                wpr_t[kt][:, c * 512:(c + 1) * 512],
                                start=(kt == 0), stop=(kt == 1))
                    ob = op.tile([128, D], F32, tag="ob", name=f"ob{m}")
                    if m % 2 == 0:
                        nc.vector.tensor_copy(ob[:], po[:])
                    else:
                        nc.scalar.copy(ob[:], po[:])
                    nc.sync.dma_start(out=out_d[m * 128:(m + 1) * 128, :],
                                      in_=ob[:])

    nc.compile()
    return nc


def _host_prep(x, w_qkv, w_proj):
    """Per-core input slices (all float32, C-contiguous)."""
    xT = [np.ascontiguousarray(x[b].T) for b in range(B)]
    in_maps = []
    for c in range(N_CORES):
        b, g = c // 4, c % 4
        heads = range(g * HPC, (g + 1) * HPC)
        wqk = np.empty((D, HPC * 2 * HD), np.float32)
        wv = np.zeros((D, HPC * (HD + 1)), np.float32)
        wpr = np.empty((HPC * HD, D), np.float32)
        for j, h in enumerate(heads):
            # layout: [q0 q1 q2 q3 k0 k1 k2 k3], 64 cols each
            wqk[:, j * HD:(j + 1) * HD] = w_qkv[:, h * HD:(h + 1) * HD]
            wqk[:, HPC * HD + j * HD:HPC * HD + (j + 1) * HD] = \
                w_qkv[:, D + h * HD:D + (h + 1) * HD]
            wv[:, j * (HD + 1):j * (HD + 1) + HD] = \
                w_qkv[:, 2 * D + h * HD:2 * D + (h + 1) * HD]
            # w_proj rows ordered to match attn head-pair packing
            wpr[j * HD:(j + 1) * HD, :] = w_proj[h * HD:(h + 1) * HD, :]
        in_maps.append({
            "xT": xT[b],
            "wqk": np.ascontiguousarray(wqk),
            "wv": np.ascontiguousarray(wv),
            "wpr": np.ascontiguousarray(wpr),
        })
    return in_maps


def get_program(block_mask: np.ndarray):
    key = np.asarray(block_mask, bool).tobytes()
    if key not in _program_cache:
        _program_cache[key] = _build_program(np.asarray(block_mask, bool))
    return _program_cache[key]


def kernel(x, w_qkv, w_proj, b_proj, block_mask):
    x = np.asarray(x, np.float32)
    w_qkv = np.asarray(w_qkv, np.float32)
    w_proj = np.asarray(w_proj, np.float32)
    b_proj = np.asarray(b_proj, np.float32)
    nc = get_program(block_mask)
    in_maps = _host_prep(x, w_qkv, w_proj)
    res = run_bass_kernel_spmd(nc, in_maps, core_ids=list(range(N_CORES)))
    out = np.empty((B, S, D), np.float32)
    for b in range(B):
        acc = res.results[4 * b]["out"].astype(np.float64)
        for g in range(1, 4):
            acc = acc + res.results[4 * b + g]["out"]
        out[b] = (acc + b_proj).astype(np.float32)
    return out
